# revision 1
# baseline (speedup 1.0000x reference)
"""Trainium2 Bass kernel for nn_ControlFlexHNN (dense_mlp).

Data-parallel across 8 NeuronCores: batch N=32768 -> 4096 rows/core.
All activations are kept feature-major ("transposed": [feature, batch])
on-chip so every matmul contracts over the partition dimension.

Host-side work (part of sharding/layout, O(N*20) or O(H^2)):
  - u = tanh(z @ Wp.T + bp) (the detached policy, tiny)
  - zu = [z, u] transposed per-core shard
  - weight layout prep (transposes / chunking)
  - final J-map: out = [s[:, DQ:], -s[:, :DQ]] and gather

Device kernel per core (B=512 batch tile, 8 tiles):
  a1 = W1 @ zT;      h1 = tanh(a1+b1); s0 = 1-h1^2
  a2 = W2 @ h1;      s1 = 1-tanh(a2+b2)^2
  f1 = Wf1 @ zuT;    g1 = tanh(f1+bf1) + (f1+bf1)*s1
  ga2 = s1 * Wh
  gh = W2.T-contract @ ga2;  ga1 = gh * s0
  f2 = Wf2 @ g1;     g2 = tanh(f2+bf2) + (f2+bf2)*s0
  sT = W1-contract @ ga1 + Wff @ g2 + bff   (accumulated in one PSUM tile)

Matmuls run as float32r (TF32-like: full fp32 storage, ~1.3e-4 rel err,
bf16-rate streaming). PSUM accumulation is fp32.
"""

import numpy as np

N = 32768
DQ = 8
D2 = 2 * DQ          # 16
A_DIM = 4
ZU = D2 + A_DIM      # 20
H = 1024
HC = H // 128        # 8 chunks
NCORES = 8
NSH = N // NCORES    # 4096 rows per core
B = 512              # batch tile (free dim of matmuls)
TILES = NSH // B     # 8

_BUILT = None


def _build(loop_n=None):
    """Build the kernel. loop_n wraps the whole 8-tile body in an on-device
    For_i loop (used only for HW timing via replication differencing)."""
    import contextlib

    import concourse.bacc as bacc
    import concourse.mybir as mybir
    from concourse import tile

    f32 = mybir.dt.float32
    f32r = mybir.dt.float32r
    Tanh = mybir.ActivationFunctionType.Tanh
    Ident = mybir.ActivationFunctionType.Identity
    mult = mybir.AluOpType.mult
    add = mybir.AluOpType.add

    nc = bacc.Bacc(None)

    zut_d = nc.dram_tensor("zut", [ZU, NSH], f32r, kind="ExternalInput")
    w1t_d = nc.dram_tensor("w1t", [D2, H], f32r, kind="ExternalInput")
    w1n_d = nc.dram_tensor("w1n", [H, D2], f32r, kind="ExternalInput")
    w2t_d = nc.dram_tensor("w2t", [H, H], f32r, kind="ExternalInput")
    w2n_d = nc.dram_tensor("w2n", [H, H], f32r, kind="ExternalInput")
    wf1t_d = nc.dram_tensor("wf1t", [ZU, H], f32r, kind="ExternalInput")
    wf2t_d = nc.dram_tensor("wf2t", [H, H], f32r, kind="ExternalInput")
    wfft_d = nc.dram_tensor("wfft", [H, D2], f32r, kind="ExternalInput")
    whc_d = nc.dram_tensor("whc", [128, HC], f32, kind="ExternalInput")
    b1c_d = nc.dram_tensor("b1c", [128, HC], f32, kind="ExternalInput")
    b2c_d = nc.dram_tensor("b2c", [128, HC], f32, kind="ExternalInput")
    bf1c_d = nc.dram_tensor("bf1c", [128, HC], f32, kind="ExternalInput")
    bf2c_d = nc.dram_tensor("bf2c", [128, HC], f32, kind="ExternalInput")
    bffc_d = nc.dram_tensor("bffc", [D2, 1], f32, kind="ExternalInput")
    st_d = nc.dram_tensor("st", [D2, NSH], f32, kind="ExternalOutput")

    with tile.TileContext(nc) as tc:
        with (
            tc.tile_pool(name="wp", bufs=1) as wp,
            tc.tile_pool(name="actp", bufs=1) as actp,
            tc.tile_pool(name="tmpp", bufs=2) as tmpp,
            tc.tile_pool(name="iop", bufs=2) as iop,
            tc.tile_pool(name="mmp", bufs=3, space="PSUM") as mmp,
            tc.tile_pool(name="accp", bufs=2, space="PSUM") as accp,
        ):
            # ---- resident weights ----
            w1t = wp.tile([D2, H], f32r)
            nc.sync.dma_start(w1t[:], w1t_d[:])
            w1n = wp.tile([128, HC, D2], f32r)
            nc.sync.dma_start(w1n[:], w1n_d.rearrange("(c p) m -> p c m", p=128))
            w2t = wp.tile([128, HC, H], f32r)
            nc.sync.dma_start(w2t[:], w2t_d.rearrange("(c p) j -> p c j", p=128))
            w2n = wp.tile([128, HC, H], f32r)
            nc.sync.dma_start(w2n[:], w2n_d.rearrange("(c p) k -> p c k", p=128))
            wf1t = wp.tile([ZU, H], f32r)
            nc.sync.dma_start(wf1t[:], wf1t_d[:])
            wf2t = wp.tile([128, HC, H], f32r)
            nc.sync.dma_start(wf2t[:], wf2t_d.rearrange("(c p) j -> p c j", p=128))
            wfft = wp.tile([128, HC, D2], f32r)
            nc.sync.dma_start(wfft[:], wfft_d.rearrange("(c p) m -> p c m", p=128))
            whc = wp.tile([128, HC], f32)
            nc.sync.dma_start(whc[:], whc_d[:])
            b1c = wp.tile([128, HC], f32)
            nc.sync.dma_start(b1c[:], b1c_d[:])
            b2c = wp.tile([128, HC], f32)
            nc.sync.dma_start(b2c[:], b2c_d[:])
            bf1c = wp.tile([128, HC], f32)
            nc.sync.dma_start(bf1c[:], bf1c_d[:])
            bf2c = wp.tile([128, HC], f32)
            nc.sync.dma_start(bf2c[:], bf2c_d[:])
            bffc = wp.tile([D2, 1], f32)
            nc.sync.dma_start(bffc[:], bffc_d[:])

            loop_cm = tc.For_i(0, loop_n, 1) if loop_n else contextlib.nullcontext()
            with loop_cm:
                _emit_body(nc, tc, tmpp, iop, actp, mmp, accp, mybir,
                           zut_d, st_d, w1t, w1n, w2t, w2n, wf1t, wf2t, wfft,
                           whc, b1c, b2c, bf1c, bf2c, bffc)

    nc.compile()
    return nc


def _build_looped(loop_n):
    return _build(loop_n=loop_n)


def _emit_body(nc, tc, tmpp, iop, actp, mmp, accp, mybir,
               zut_d, st_d, w1t, w1n, w2t, w2n, wf1t, wf2t, wfft,
               whc, b1c, b2c, bf1c, bf2c, bffc):
    f32 = mybir.dt.float32
    f32r = mybir.dt.float32r
    Tanh = mybir.ActivationFunctionType.Tanh
    Ident = mybir.ActivationFunctionType.Identity
    mult = mybir.AluOpType.mult
    add = mybir.AluOpType.add

    Copy = mybir.ActivationFunctionType.Copy

    for t in range(TILES):
        sl = slice(t * B, (t + 1) * B)
        zut = iop.tile([ZU, B], f32r, tag="zut", name=f"zut_{t}")
        nc.sync.dma_start(zut[:], zut_d[:, sl])
        zt = zut[0:D2, :]

        h1 = actp.tile([128, HC, B], f32r, tag="h1", name=f"h1_{t}")
        s0 = actp.tile([128, HC, B], f32, tag="s0", name=f"s0_{t}")
        ga2 = actp.tile([128, HC, B], f32r, tag="ga2", name=f"ga2_{t}")
        g1 = actp.tile([128, HC, B], f32r, tag="g1", name=f"g1_{t}")

        # ---- layer A: a1 -> h1, s0 ----
        for j in range(HC):
            pa = mmp.tile([128, B], f32, tag="mm", name=f"pa_{t}_{j}")
            nc.tensor.matmul(pa[:], w1t[:, j * 128:(j + 1) * 128], zt,
                             start=True, stop=True)
            nc.scalar.activation(h1[:, j, :], pa[:], Tanh,
                                 bias=b1c[:, j:j + 1])
            nc.vector.tensor_tensor(out=s0[:, j, :], in0=h1[:, j, :],
                                    in1=h1[:, j, :], op=mult)
            nc.vector.tensor_scalar(out=s0[:, j, :], in0=s0[:, j, :],
                                    scalar1=-1.0, scalar2=1.0,
                                    op0=mult, op1=add)

        # ---- layer B+C: a2 -> s1; f1 -> g1; ga2 ----
        for j in range(HC):
            pb = mmp.tile([128, B], f32, tag="mm", name=f"pb_{t}_{j}")
            for k in range(HC):
                nc.tensor.matmul(pb[:], w2t[:, k, j * 128:(j + 1) * 128],
                                 h1[:, k, :], start=(k == 0), stop=(k == 7))
            h2t = tmpp.tile([128, B], f32, tag="h2t", name=f"h2t_{t}_{j}")
            nc.scalar.activation(h2t[:], pb[:], Tanh, bias=b2c[:, j:j + 1])
            s1t = tmpp.tile([128, B], f32, tag="s1t", name=f"s1t_{t}_{j}")
            nc.vector.tensor_tensor(out=s1t[:], in0=h2t[:], in1=h2t[:], op=mult)
            # s1 = 1 - h2^2 on ACT (Identity: -1*x + 1)
            nc.scalar.activation(s1t[:], s1t[:], Ident, bias=1.0, scale=-1.0)

            pf = mmp.tile([128, B], f32, tag="mm", name=f"pf_{t}_{j}")
            nc.tensor.matmul(pf[:], wf1t[:, j * 128:(j + 1) * 128], zut[:],
                             start=True, stop=True)
            th = tmpp.tile([128, B], f32, tag="th", name=f"th_{t}_{j}")
            nc.scalar.activation(th[:], pf[:], Tanh, bias=bf1c[:, j:j + 1])
            t1 = tmpp.tile([128, B], f32, tag="t1", name=f"t1_{t}_{j}")
            nc.vector.tensor_scalar_add(t1[:], pf[:], bf1c[:, j:j + 1])
            prod = tmpp.tile([128, B], f32, tag="prod", name=f"prod_{t}_{j}")
            nc.vector.tensor_tensor(out=prod[:], in0=t1[:], in1=s1t[:], op=mult)
            nc.gpsimd.tensor_tensor(out=g1[:, j, :], in0=th[:], in1=prod[:], op=add)
            # ga2 = s1 * Wh on ACT (Copy with per-partition scale)
            nc.scalar.activation(ga2[:, j, :], s1t[:], Copy,
                                 scale=whc[:, j:j + 1])

        # ---- layer D: gh -> ga1 -> dH accum (head MMs pipelined 1 behind) ----
        ps = accp.tile([D2, B], f32, tag="acc", name=f"ps_{t}")
        ga1s = []
        for k in range(HC):
            pg = mmp.tile([128, B], f32, tag="mm", name=f"pg_{t}_{k}")
            for j in range(HC):
                nc.tensor.matmul(pg[:], w2n[:, j, k * 128:(k + 1) * 128],
                                 ga2[:, j, :], start=(j == 0), stop=(j == 7))
            ga1 = tmpp.tile([128, B], f32r, tag="gr", name=f"ga1_{t}_{k}")
            nc.vector.tensor_tensor(out=ga1[:], in0=pg[:], in1=s0[:, k, :], op=mult)
            ga1s.append(ga1)
            if k >= 1:
                nc.tensor.matmul(ps[:], w1n[:, k - 1, :], ga1s[k - 1][:],
                                 start=(k == 1), stop=False)
        nc.tensor.matmul(ps[:], w1n[:, HC - 1, :], ga1s[HC - 1][:],
                         start=False, stop=False)

        # ---- layer E+F: f2 -> g2 -> head accum (pipelined 1 behind) ----
        g2s = []
        for j in range(HC):
            pf2 = mmp.tile([128, B], f32, tag="mm", name=f"pf2_{t}_{j}")
            for k in range(HC):
                nc.tensor.matmul(pf2[:], wf2t[:, k, j * 128:(j + 1) * 128],
                                 g1[:, k, :], start=(k == 0), stop=(k == 7))
            th2 = tmpp.tile([128, B], f32, tag="th", name=f"th2_{t}_{j}")
            nc.scalar.activation(th2[:], pf2[:], Tanh, bias=bf2c[:, j:j + 1])
            t2 = tmpp.tile([128, B], f32, tag="t1", name=f"t2_{t}_{j}")
            nc.vector.tensor_scalar_add(t2[:], pf2[:], bf2c[:, j:j + 1])
            prod2 = tmpp.tile([128, B], f32, tag="prod", name=f"prod2_{t}_{j}")
            nc.vector.tensor_tensor(out=prod2[:], in0=t2[:], in1=s0[:, j, :], op=mult)
            g2t = tmpp.tile([128, B], f32r, tag="gr", name=f"g2t_{t}_{j}")
            nc.gpsimd.tensor_tensor(out=g2t[:], in0=th2[:], in1=prod2[:], op=add)
            g2s.append(g2t)
            if j >= 1:
                nc.tensor.matmul(ps[:], wfft[:, j - 1, :], g2s[j - 1][:],
                                 start=False, stop=False)
        nc.tensor.matmul(ps[:], wfft[:, HC - 1, :], g2s[HC - 1][:],
                         start=False, stop=True)

        sout = iop.tile([D2, B], f32, tag="sout", name=f"sout_{t}")
        nc.scalar.activation(sout[:], ps[:], Ident, bias=bffc[:, 0:1])
        nc.sync.dma_start(st_d[:, sl], sout[:])


def _prep_inputs(t, z, W1, b1, W2, b2, Wh, bh, Wf1, bf1, Wf2, bf2, Wff, bff,
                 Wp, bp):
    f = np.float32
    z = np.asarray(z, f)
    u = np.tanh(z @ np.asarray(Wp, f).T + np.asarray(bp, f))
    zu = np.concatenate([z, u], axis=1)          # [N, 20]

    def c(x):
        return np.ascontiguousarray(np.asarray(x, f))

    shared = {
        "w1t": c(np.asarray(W1, f).T),
        "w1n": c(W1),
        "w2t": c(np.asarray(W2, f).T),
        "w2n": c(W2),
        "wf1t": c(np.asarray(Wf1, f).T),
        "wf2t": c(np.asarray(Wf2, f).T),
        "wfft": c(np.asarray(Wff, f).T),
        "whc": c(np.asarray(Wh, f).reshape(HC, 128).T),
        "b1c": c(np.asarray(b1, f).reshape(HC, 128).T),
        "b2c": c(np.asarray(b2, f).reshape(HC, 128).T),
        "bf1c": c(np.asarray(bf1, f).reshape(HC, 128).T),
        "bf2c": c(np.asarray(bf2, f).reshape(HC, 128).T),
        "bffc": c(np.asarray(bff, f).reshape(D2, 1)),
    }
    in_maps = []
    for r in range(NCORES):
        m = dict(shared)
        m["zut"] = c(zu[r * NSH:(r + 1) * NSH].T)
        in_maps.append(m)
    return in_maps


def _postprocess(results):
    outs = []
    for r in range(NCORES):
        s = results[r]["st"].T                    # [NSH, 16]
        outs.append(np.concatenate([s[:, DQ:], -s[:, :DQ]], axis=1))
    return np.ascontiguousarray(np.concatenate(outs, axis=0).astype(np.float32))


def kernel(**inputs):
    global _BUILT
    from concourse.bass_utils import run_bass_kernel_spmd

    if _BUILT is None:
        _BUILT = _build()
    in_maps = _prep_inputs(**inputs)
    res = run_bass_kernel_spmd(_BUILT, in_maps, list(range(NCORES)))
    return _postprocess(res.results)



# revision 13
# speedup vs baseline: 3.1126x; 3.1126x over previous
"""Trainium2 Bass kernel for nn_ControlFlexHNN (dense_mlp).

Data-parallel across 8 NeuronCores: batch N=32768 -> 4096 rows/core.
All activations are kept feature-major ([feature, batch]) on-chip so
every matmul contracts over the partition dimension.

Host-side work (part of sharding/layout, O(N*20) or O(H^2)):
  - u = tanh(z @ Wp.T + bp) (the detached policy, tiny)
  - zu = [z, u] transposed per-core shard
  - weight layout prep; Wh is folded into the backward W2 copy:
      w2w[j,k] = W2[j,k] * Wh[j]  so  gh = w2w^T-contract @ s1
  - final J-map: out = [s[:, DQ:], -s[:, :DQ]] and gather

Device kernel per core (B=512 batch tile, 8 tiles), per tile t:
  BC: pb_j = W2 @ h1           ; h2 = tanh(pb+b2); s1_j = 1-h2^2
      pf_j = Wf1 @ zu          ; th = tanh(pf+bf1)
      prod = (pf+bf1)*s1 (fused DVE affine_mul) ; g1_j = th+prod (Pool)
  seam/D: pa_{t+1,j} = W1 @ z_{t+1} ; h1_{t+1} = tanh(pa+b1)  (hoisted)
      pg_k = w2w^T @ s1        ; s0_k = 1-h1_t^2 (lazy, DVE)
      ga1_k = pg*s0            ; ps += W1n @ ga1 (pipelined)
  EF: pf2_j = Wf2 @ g1         ; th2 = tanh(pf2+bf2)
      prod2 = (pf2+bf2)*s0 (fused) ; g2 = th2+prod2 (Pool)
      ps += Wff @ g2 (pipelined) ; sout = ps + bff -> DMA

Matmuls stream f32r (stationary) x f32r/bf16 (moving) at 1 col/cycle.
h1 and g1 live in SBUF as bf16 (moving-only operands); s1 is f32r,
s0 f32. PSUM accumulation is fp32.
"""

import numpy as np

N = 32768
DQ = 8
D2 = 2 * DQ          # 16
A_DIM = 4
ZU = D2 + A_DIM      # 20
H = 1024
HC = H // 128        # 8 chunks
NCORES = 8
NSH = N // NCORES    # 4096 rows per core
B = 512              # batch tile (free dim of matmuls)
TILES = NSH // B     # 8

_BUILT = None


def _build(loop_n=None):
    """Build the kernel. loop_n wraps the whole 8-tile body in an on-device
    For_i loop (used only for HW timing via replication differencing)."""
    import contextlib

    import concourse.bacc as bacc
    import concourse.mybir as mybir
    from concourse import tile

    f32 = mybir.dt.float32
    f32r = mybir.dt.float32r
    bf16 = mybir.dt.bfloat16
    fp8 = mybir.dt.float8e4

    nc = bacc.Bacc(None)

    zut_d = nc.dram_tensor("zut", [ZU, NSH], f32r, kind="ExternalInput")
    w1t_d = nc.dram_tensor("w1t", [D2, H], f32r, kind="ExternalInput")
    w1n_d = nc.dram_tensor("w1n", [H, D2], f32r, kind="ExternalInput")
    w2t_d = nc.dram_tensor("w2t", [H, H], bf16, kind="ExternalInput")
    w2w_d = nc.dram_tensor("w2w", [H, H], fp8, kind="ExternalInput")
    wf1t_d = nc.dram_tensor("wf1t", [ZU, H], f32r, kind="ExternalInput")
    wf2t_d = nc.dram_tensor("wf2t", [H, H], bf16, kind="ExternalInput")
    wfft_d = nc.dram_tensor("wfft", [H, D2], f32r, kind="ExternalInput")
    b1c_d = nc.dram_tensor("b1c", [128, HC], f32, kind="ExternalInput")
    b2c_d = nc.dram_tensor("b2c", [128, HC], f32, kind="ExternalInput")
    bf1c_d = nc.dram_tensor("bf1c", [128, HC], f32, kind="ExternalInput")
    bf2c_d = nc.dram_tensor("bf2c", [128, HC], f32, kind="ExternalInput")
    bffc_d = nc.dram_tensor("bffc", [D2, 1], f32, kind="ExternalInput")
    st_d = nc.dram_tensor("st", [D2, NSH], f32, kind="ExternalOutput")

    with tile.TileContext(nc) as tc:
        with (
            tc.tile_pool(name="wp", bufs=1) as wp,
            tc.tile_pool(name="actp", bufs=1) as actp,
            tc.tile_pool(name="h1p", bufs=2) as h1p,
            tc.tile_pool(name="tmpp", bufs=3) as tmpp,
            tc.tile_pool(name="tmp2", bufs=2) as tmp2,
            tc.tile_pool(name="iop", bufs=3) as iop,
            tc.tile_pool(name="outp", bufs=2) as outp,
            tc.tile_pool(name="mmp", bufs=3, space="PSUM") as mmp,
            tc.tile_pool(name="smp", bufs=3, space="PSUM") as smp,
            tc.tile_pool(name="accp", bufs=2, space="PSUM") as accp,
        ):
            # ---- tile-0/1 inputs first (prologue depends on them), then
            # resident weights, ordered/split so compute starts early:
            # layer-1 weights land first (prologue), then w2t in 128-column
            # blocks (BC consumes block j at ~2us intervals), then w2w (D),
            # then wf2t (EF).
            zut0 = wp.tile([ZU, B], f32r)
            nc.sync.dma_start(zut0[:], zut_d[:, 0:B])
            zut1 = wp.tile([ZU, B], f32r)
            nc.sync.dma_start(zut1[:], zut_d[:, B:2 * B])
            w1t = wp.tile([D2, H], f32r)
            nc.sync.dma_start(w1t[:], w1t_d[:])
            b1c = wp.tile([128, HC], f32)
            nc.sync.dma_start(b1c[:], b1c_d[:])
            b2c = wp.tile([128, HC], f32)
            nc.sync.dma_start(b2c[:], b2c_d[:])
            wf1t = wp.tile([ZU, H], f32r)
            nc.sync.dma_start(wf1t[:], wf1t_d[:])
            bf1c = wp.tile([128, HC], f32)
            nc.sync.dma_start(bf1c[:], bf1c_d[:])
            w2t = wp.tile([128, HC, H], bf16)
            for blk in range(HC):
                sl = slice(blk * 128, (blk + 1) * 128)
                nc.sync.dma_start(
                    w2t[:, :, sl],
                    w2t_d[:, sl].rearrange("(c p) j -> p c j", p=128))
            w2w = wp.tile([128, HC, H], fp8)
            for blk in range(HC):
                sl = slice(blk * 128, (blk + 1) * 128)
                nc.sync.dma_start(
                    w2w[:, :, sl],
                    w2w_d[:, sl].rearrange("(c p) k -> p c k", p=128))
            w1n = wp.tile([128, HC, D2], f32r)
            nc.sync.dma_start(w1n[:], w1n_d.rearrange("(c p) m -> p c m", p=128))
            wf2t = wp.tile([128, HC, H], bf16)
            for blk in range(HC):
                sl = slice(blk * 128, (blk + 1) * 128)
                nc.sync.dma_start(
                    wf2t[:, :, sl],
                    wf2t_d[:, sl].rearrange("(c p) j -> p c j", p=128))
            bf2c = wp.tile([128, HC], f32)
            nc.sync.dma_start(bf2c[:], bf2c_d[:])
            wfft = wp.tile([128, HC, D2], f32r)
            nc.sync.dma_start(wfft[:], wfft_d.rearrange("(c p) m -> p c m", p=128))
            bffc = wp.tile([D2, 1], f32)
            nc.sync.dma_start(bffc[:], bffc_d[:])

            weights = (w1t, w1n, w2t, w2w, wf1t, wf2t, wfft,
                       b1c, b2c, bf1c, bf2c, bffc)
            pools = (actp, h1p, tmpp, tmp2, iop, outp, mmp, smp, accp)

            loop_cm = tc.For_i(0, loop_n, 1) if loop_n else contextlib.nullcontext()
            with loop_cm:
                _emit_body(nc, tc, pools, weights, mybir, zut_d, st_d,
                           zut0, zut1)

    nc.compile()
    return nc


def _build_looped(loop_n):
    return _build(loop_n=loop_n)


def _emit_body(nc, tc, pools, weights, mybir, zut_d, st_d, zut0, zut1):
    (actp, h1p, tmpp, tmp2, iop, outp, mmp, smp, accp) = pools
    (w1t, w1n, w2t, w2w, wf1t, wf2t, wfft,
     b1c, b2c, bf1c, bf2c, bffc) = weights

    f32 = mybir.dt.float32
    f32r = mybir.dt.float32r
    bf16 = mybir.dt.bfloat16
    fp8 = mybir.dt.float8e4
    DoubleRow = mybir.MatmulPerfMode.DoubleRow
    Copy = mybir.ActivationFunctionType.Copy
    Tanh = mybir.ActivationFunctionType.Tanh
    Ident = mybir.ActivationFunctionType.Identity
    mult = mybir.AluOpType.mult
    add = mybir.AluOpType.add

    def dma_zut(t):
        z = iop.tile([ZU, B], f32r, tag="zut", name=f"zut_{t}")
        nc.sync.dma_start(z[:], zut_d[:, t * B:(t + 1) * B])
        return z

    def emit_A_chunk(t, j, zts, h1t):
        """pa_{t,j} = W1 @ z_t ; h1_t[:,j,:] = tanh(pa + b1). PE + ACT."""
        pa = smp.tile([128, B], f32, tag="sm", name=f"pa_{t}_{j}")
        nc.tensor.matmul(pa[:], w1t[:, j * 128:(j + 1) * 128], zts[0:D2, :],
                         start=True, stop=True)
        nc.scalar.activation(h1t[:, j, :], pa[:], Tanh, bias=b1c[:, j:j + 1])

    def emit_pb(t, j, h1t):
        """a2 pre-activation matmul group for chunk j (8 accumulating mms)."""
        pb = mmp.tile([128, B], f32, tag="mm", name=f"pb_{t}_{j}")
        for k in range(HC):
            nc.tensor.matmul(pb[:], w2t[:, k, j * 128:(j + 1) * 128],
                             h1t[:, k, :], start=(k == 0), stop=(k == 7))
        return pb

    # ---- resident activation tensors ----
    s1 = actp.tile([128, HC, B], f32r, tag="s1", name="s1")
    s18 = actp.tile([128, HC, B], fp8, tag="s18", name="s18")
    s0 = actp.tile([128, HC, B], f32, tag="s0", name="s0")
    g1 = actp.tile([128, HC, B], bf16, tag="g1", name="g1")

    # ---- prologue: tile 0 layer-1 forward (zut0/zut1 pre-DMA'd) ----
    zuts = {0: zut0, 1: zut1}
    h1s = {0: h1p.tile([128, HC, B], bf16, tag="h1", name="h1_0")}
    for j in range(HC):
        emit_A_chunk(0, j, zuts[0], h1s[0])

    pb0_next = None
    for t in range(TILES):
        sl = slice(t * B, (t + 1) * B)
        zut = zuts.pop(t)
        h1 = h1s.pop(t)
        if t + 2 < TILES:
            zuts[t + 2] = dma_zut(t + 2)
        if t + 1 < TILES:
            h1s[t + 1] = h1p.tile([128, HC, B], bf16, tag="h1",
                                  name=f"h1_{t + 1}")

        # ---- BC: a2 -> s1 ; f1 -> g1 ----
        for j in range(HC):
            pb = pb0_next if (j == 0 and pb0_next is not None) \
                else emit_pb(t, j, h1)
            h2 = tmp2.tile([128, B], bf16, tag="h2", name=f"h2_{t}_{j}")
            nc.scalar.activation(h2[:], pb[:], Tanh, bias=b2c[:, j:j + 1])
            nc.vector.tensor_tensor(out=h2[:], in0=h2[:], in1=h2[:], op=mult)
            nc.vector.tensor_scalar(out=s1[:, j, :], in0=h2[:],
                                    scalar1=-1.0, scalar2=1.0,
                                    op0=mult, op1=add)
            nc.vector.tensor_scalar(out=s18[:, j, :], in0=h2[:],
                                    scalar1=-128.0, scalar2=128.0,
                                    op0=mult, op1=add)

            pf = smp.tile([128, B], f32, tag="sm", name=f"pf_{t}_{j}")
            nc.tensor.matmul(pf[:], wf1t[:, j * 128:(j + 1) * 128], zut[:],
                             start=True, stop=True)
            th = tmpp.tile([128, B], f32, tag="th", name=f"th_{t}_{j}")
            nc.scalar.activation(th[:], pf[:], Tanh, bias=bf1c[:, j:j + 1])
            prod = tmpp.tile([128, B], f32, tag="prod", name=f"prod_{t}_{j}")
            acc = tmp2.tile([128, 1], f32, tag="acc", name=f"acc_{t}_{j}")
            nc.vector.affine_mul_reduce(prod[:], acc[:], pf[:], s1[:, j, :],
                                        1.0, bf1c[:, j:j + 1])
            nc.gpsimd.tensor_tensor(out=g1[:, j, :], in0=th[:], in1=prod[:],
                                    op=add)
        pb0_next = None

        # ---- seam: start next tile's layer-1 (covers s1_7 ACT/DVE latency) --
        if t + 1 < TILES:
            for j in range(2):
                emit_A_chunk(t + 1, j, zuts[t + 1], h1s[t + 1])

        # ---- D: gh -> ga1 -> dH accum ; lazy s0 ; rest of next layer-1 ----
        # gh runs as fp8 DoubleRow: 4 matmuls of K=256 (chunk pairs). The
        # 2^22 fp8 scaling (w2w x2^14, s1 x2^8) is undone host-side in w1n.
        ps = accp.tile([D2, B], f32, tag="acc", name=f"ps_{t}")
        ga1s = []
        for k in range(HC):
            pg = mmp.tile([128, B], f32, tag="mm", name=f"pg_{t}_{k}")
            for pr in range(HC // 2):
                nc.tensor.matmul(pg[:],
                                 w2w[:, 2 * pr:2 * pr + 2,
                                     k * 128:(k + 1) * 128],
                                 s18[:, 2 * pr:2 * pr + 2, :],
                                 start=(pr == 0), stop=(pr == 3),
                                 perf_mode=DoubleRow)
            sq = tmp2.tile([128, B], bf16, tag="sq", name=f"sq_{t}_{k}")
            nc.vector.tensor_tensor(out=sq[:], in0=h1[:, k, :], in1=h1[:, k, :],
                                    op=mult)
            nc.vector.tensor_scalar(out=s0[:, k, :], in0=sq[:],
                                    scalar1=-1.0, scalar2=1.0,
                                    op0=mult, op1=add)
            ga1 = tmp2.tile([128, B], f32r, tag="ga1", name=f"ga1_{t}_{k}")
            eng = nc.vector if k % 2 == 0 else nc.gpsimd
            eng.tensor_tensor(out=ga1[:], in0=pg[:], in1=s0[:, k, :],
                              op=mult)
            ga1s.append(ga1)
            if k >= 2:
                nc.tensor.matmul(ps[:], w1n[:, k - 2, :], ga1s[k - 2][:],
                                 start=(k == 2), stop=False)
            if t + 1 < TILES and k < HC - 2:
                emit_A_chunk(t + 1, 2 + k, zuts[t + 1], h1s[t + 1])

        # ---- EF: f2 -> g2 -> head accum (pipelined 1 behind; g2 on DVE).
        # pf2_0 is emitted before the two trailing dH accumulations so the
        # PE isn't waiting on ga1_7's elementwise chain.
        g2s = []
        for j in range(HC):
            pf2 = mmp.tile([128, B], f32, tag="mm", name=f"pf2_{t}_{j}")
            for k in range(HC):
                nc.tensor.matmul(pf2[:], wf2t[:, k, j * 128:(j + 1) * 128],
                                 g1[:, k, :], start=(k == 0), stop=(k == 7))
            if j == 0:
                nc.tensor.matmul(ps[:], w1n[:, HC - 2, :], ga1s[HC - 2][:],
                                 start=False, stop=False)
                nc.tensor.matmul(ps[:], w1n[:, HC - 1, :], ga1s[HC - 1][:],
                                 start=False, stop=False)
            th2 = tmp2.tile([128, B], f32, tag="th", name=f"th2_{t}_{j}")
            nc.scalar.activation(th2[:], pf2[:], Tanh, bias=bf2c[:, j:j + 1])
            prod2 = tmpp.tile([128, B], f32, tag="prod", name=f"prod2_{t}_{j}")
            acc2 = tmp2.tile([128, 1], f32, tag="acc", name=f"acc2_{t}_{j}")
            nc.vector.affine_mul_reduce(prod2[:], acc2[:], pf2[:], s0[:, j, :],
                                        1.0, bf2c[:, j:j + 1])
            g2 = tmp2.tile([128, B], f32r, tag="g2", name=f"g2_{t}_{j}")
            nc.vector.tensor_tensor(out=g2[:], in0=th2[:], in1=prod2[:], op=add)
            g2s.append(g2)
            if j >= 1:
                nc.tensor.matmul(ps[:], wfft[:, j - 1, :], g2s[j - 1][:],
                                 start=False, stop=False)

        # hoist next tile's first a2 matmul group ahead of the trailing head
        # accumulation so the PE isn't waiting on g2_7's elementwise chain
        if t + 1 < TILES:
            pb0_next = emit_pb(t + 1, 0, h1s[t + 1])
        nc.tensor.matmul(ps[:], wfft[:, HC - 1, :], g2s[HC - 1][:],
                         start=False, stop=True)

        sout = outp.tile([D2, B], f32, tag="sout", name=f"sout_{t}")
        nc.scalar.activation(sout[:], ps[:], Ident, bias=bffc[:, 0:1])
        nc.sync.dma_start(st_d[:, sl], sout[:])


def _prep_inputs(t, z, W1, b1, W2, b2, Wh, bh, Wf1, bf1, Wf2, bf2, Wff, bff,
                 Wp, bp):
    f = np.float32
    z = np.asarray(z, f)
    u = np.tanh(z @ np.asarray(Wp, f).T + np.asarray(bp, f))
    zu = np.concatenate([z, u], axis=1)          # [N, 20]

    def c(x):
        return np.ascontiguousarray(np.asarray(x, f))

    import ml_dtypes
    bf16 = ml_dtypes.bfloat16
    fp8 = ml_dtypes.float8_e4m3
    W2 = np.asarray(W2, f)
    wh = np.asarray(Wh, f).reshape(-1, 1)        # [H, 1]

    # gh GEMM runs in fp8 (DoubleRow): scale W2w by 2^14 and s1 by 2^7 on
    # chip (e4m3 max finite 240); the 2^-21 descale is folded into w1n here.
    shared = {
        "w1t": c(np.asarray(W1, f).T),
        "w1n": c(np.asarray(W1, f) * np.float32(2.0 ** -21)),
        "w2t": np.ascontiguousarray(W2.T.astype(bf16)),
        "w2w": np.ascontiguousarray(
            ((W2 * wh) * np.float32(2.0 ** 14)).astype(fp8)),
        "wf1t": c(np.asarray(Wf1, f).T),
        "wf2t": np.ascontiguousarray(np.asarray(Wf2, f).T.astype(bf16)),
        "wfft": c(np.asarray(Wff, f).T),
        "b1c": c(np.asarray(b1, f).reshape(HC, 128).T),
        "b2c": c(np.asarray(b2, f).reshape(HC, 128).T),
        "bf1c": c(np.asarray(bf1, f).reshape(HC, 128).T),
        "bf2c": c(np.asarray(bf2, f).reshape(HC, 128).T),
        "bffc": c(np.asarray(bff, f).reshape(D2, 1)),
    }
    in_maps = []
    for r in range(NCORES):
        m = dict(shared)
        m["zut"] = c(zu[r * NSH:(r + 1) * NSH].T)
        in_maps.append(m)
    return in_maps


def _postprocess(results):
    outs = []
    for r in range(NCORES):
        s = results[r]["st"].T                    # [NSH, 16]
        outs.append(np.concatenate([s[:, DQ:], -s[:, :DQ]], axis=1))
    return np.ascontiguousarray(np.concatenate(outs, axis=0).astype(np.float32))


def kernel(**inputs):
    global _BUILT
    from concourse.bass_utils import run_bass_kernel_spmd

    if _BUILT is None:
        _BUILT = _build()
    in_maps = _prep_inputs(**inputs)
    res = run_bass_kernel_spmd(_BUILT, in_maps, list(range(NCORES)))
    return _postprocess(res.results)


# revision 14
# speedup vs baseline: 15.0478x; 4.8344x over previous
"""Trainium2 Bass kernel for nn_ControlFlexHNN (dense_mlp).

Data-parallel across 8 NeuronCores: batch N=32768 -> 4096 rows/core.
All activations are kept feature-major ([feature, batch]) on-chip so
every matmul contracts over the partition dimension.

Host-side work (part of sharding/layout, O(N*20) or O(H^2)):
  - u = tanh(z @ Wp.T + bp) (the detached policy, tiny)
  - zu = [z, u] transposed per-core shard
  - weight layout prep; Wh is folded into the backward W2 copy:
      w2w[j,k] = W2[j,k] * Wh[j]  so  gh = w2w^T-contract @ s1
  - final J-map: out = [s[:, DQ:], -s[:, :DQ]] and gather

Device kernel per core (B=512 batch tile, 8 tiles), per tile t:
  BC: pb_j = W2 @ h1           ; h2 = tanh(pb+b2); s1_j = 1-h2^2
      pf_j = Wf1 @ zu          ; th = tanh(pf+bf1)
      prod = (pf+bf1)*s1 (fused DVE affine_mul) ; g1_j = th+prod (Pool)
  seam/D: pa_{t+1,j} = W1 @ z_{t+1} ; h1_{t+1} = tanh(pa+b1)  (hoisted)
      pg_k = w2w^T @ s1        ; s0_k = 1-h1_t^2 (lazy, DVE)
      ga1_k = pg*s0            ; ps += W1n @ ga1 (pipelined)
  EF: pf2_j = Wf2 @ g1         ; th2 = tanh(pf2+bf2)
      prod2 = (pf2+bf2)*s0 (fused) ; g2 = th2+prod2 (Pool)
      ps += Wff @ g2 (pipelined) ; sout = ps + bff -> DMA

Matmuls stream f32r (stationary) x f32r/bf16 (moving) at 1 col/cycle.
h1 and g1 live in SBUF as bf16 (moving-only operands); s1 is f32r,
s0 f32. PSUM accumulation is fp32.
"""

import numpy as np

N = 32768
DQ = 8
D2 = 2 * DQ          # 16
A_DIM = 4
ZU = D2 + A_DIM      # 20
H = 1024
HC = H // 128        # 8 chunks
NCORES = 8
NSH = N // NCORES    # 4096 rows per core
B = 512              # batch tile (free dim of matmuls)
TILES = NSH // B     # 8

_BUILT = None


def _build(loop_n=None):
    """Build the kernel. loop_n wraps the whole 8-tile body in an on-device
    For_i loop (used only for HW timing via replication differencing)."""
    import contextlib

    import concourse.bacc as bacc
    import concourse.mybir as mybir
    from concourse import tile

    f32 = mybir.dt.float32
    f32r = mybir.dt.float32r
    bf16 = mybir.dt.bfloat16
    fp8 = mybir.dt.float8e4

    nc = bacc.Bacc(None)

    zut_d = nc.dram_tensor("zut", [ZU, NSH], f32r, kind="ExternalInput")
    w1t_d = nc.dram_tensor("w1t", [D2, H], f32r, kind="ExternalInput")
    w1n_d = nc.dram_tensor("w1n", [H, D2], f32r, kind="ExternalInput")
    w2t_d = nc.dram_tensor("w2t", [H, H], bf16, kind="ExternalInput")
    w2w_d = nc.dram_tensor("w2w", [H, H], fp8, kind="ExternalInput")
    wf1t_d = nc.dram_tensor("wf1t", [ZU, H], f32r, kind="ExternalInput")
    wf2t_d = nc.dram_tensor("wf2t", [H, H], bf16, kind="ExternalInput")
    wfft_d = nc.dram_tensor("wfft", [H, D2], f32r, kind="ExternalInput")
    b1c_d = nc.dram_tensor("b1c", [128, HC], f32, kind="ExternalInput")
    b2c_d = nc.dram_tensor("b2c", [128, HC], f32, kind="ExternalInput")
    bf1c_d = nc.dram_tensor("bf1c", [128, HC], f32, kind="ExternalInput")
    bf2c_d = nc.dram_tensor("bf2c", [128, HC], f32, kind="ExternalInput")
    bffc_d = nc.dram_tensor("bffc", [D2, 1], f32, kind="ExternalInput")
    st_d = nc.dram_tensor("st", [D2, NSH], f32, kind="ExternalOutput")

    with tile.TileContext(nc) as tc:
        with (
            tc.tile_pool(name="wp", bufs=1) as wp,
            tc.tile_pool(name="actp", bufs=1) as actp,
            tc.tile_pool(name="h1p", bufs=2) as h1p,
            tc.tile_pool(name="tmpp", bufs=3) as tmpp,
            tc.tile_pool(name="tmp2", bufs=2) as tmp2,
            tc.tile_pool(name="iop", bufs=3) as iop,
            tc.tile_pool(name="outp", bufs=2) as outp,
            tc.tile_pool(name="mmp", bufs=3, space="PSUM") as mmp,
            tc.tile_pool(name="smp", bufs=3, space="PSUM") as smp,
            tc.tile_pool(name="accp", bufs=2, space="PSUM") as accp,
        ):
            # ---- tile-0/1 inputs first (prologue depends on them), then
            # resident weights, ordered/split so compute starts early:
            # layer-1 weights land first (prologue), then w2t in 128-column
            # blocks (BC consumes block j at ~2us intervals), then w2w (D),
            # then wf2t (EF).
            zut0 = wp.tile([ZU, B], f32r)
            nc.sync.dma_start(zut0[:], zut_d[:, 0:B])
            zut1 = wp.tile([ZU, B], f32r)
            nc.sync.dma_start(zut1[:], zut_d[:, B:2 * B])
            w1t = wp.tile([D2, H], f32r)
            nc.sync.dma_start(w1t[:], w1t_d[:])
            b1c = wp.tile([128, HC], f32)
            nc.sync.dma_start(b1c[:], b1c_d[:])
            b2c = wp.tile([128, HC], f32)
            nc.sync.dma_start(b2c[:], b2c_d[:])
            wf1t = wp.tile([ZU, H], f32r)
            nc.sync.dma_start(wf1t[:], wf1t_d[:])
            bf1c = wp.tile([128, HC], f32)
            nc.sync.dma_start(bf1c[:], bf1c_d[:])
            w2t = wp.tile([128, HC, H], bf16)
            for blk in range(HC):
                sl = slice(blk * 128, (blk + 1) * 128)
                nc.sync.dma_start(
                    w2t[:, :, sl],
                    w2t_d[:, sl].rearrange("(c p) j -> p c j", p=128))
            w2w = wp.tile([128, HC, H], fp8)
            for blk in range(HC):
                sl = slice(blk * 128, (blk + 1) * 128)
                nc.sync.dma_start(
                    w2w[:, :, sl],
                    w2w_d[:, sl].rearrange("(c p) k -> p c k", p=128))
            w1n = wp.tile([128, HC, D2], f32r)
            nc.sync.dma_start(w1n[:], w1n_d.rearrange("(c p) m -> p c m", p=128))
            wf2t = wp.tile([128, HC, H], bf16)
            for blk in range(HC):
                sl = slice(blk * 128, (blk + 1) * 128)
                nc.sync.dma_start(
                    wf2t[:, :, sl],
                    wf2t_d[:, sl].rearrange("(c p) j -> p c j", p=128))
            bf2c = wp.tile([128, HC], f32)
            nc.sync.dma_start(bf2c[:], bf2c_d[:])
            wfft = wp.tile([128, HC, D2], f32r)
            nc.sync.dma_start(wfft[:], wfft_d.rearrange("(c p) m -> p c m", p=128))
            bffc = wp.tile([D2, 1], f32)
            nc.sync.dma_start(bffc[:], bffc_d[:])

            weights = (w1t, w1n, w2t, w2w, wf1t, wf2t, wfft,
                       b1c, b2c, bf1c, bf2c, bffc)
            pools = (actp, h1p, tmpp, tmp2, iop, outp, mmp, smp, accp)

            loop_cm = tc.For_i(0, loop_n, 1) if loop_n else contextlib.nullcontext()
            with loop_cm:
                _emit_body(nc, tc, pools, weights, mybir, zut_d, st_d,
                           zut0, zut1)

    nc.compile()
    return nc


def _build_looped(loop_n):
    return _build(loop_n=loop_n)


def _emit_body(nc, tc, pools, weights, mybir, zut_d, st_d, zut0, zut1):
    (actp, h1p, tmpp, tmp2, iop, outp, mmp, smp, accp) = pools
    (w1t, w1n, w2t, w2w, wf1t, wf2t, wfft,
     b1c, b2c, bf1c, bf2c, bffc) = weights

    f32 = mybir.dt.float32
    f32r = mybir.dt.float32r
    bf16 = mybir.dt.bfloat16
    fp8 = mybir.dt.float8e4
    DoubleRow = mybir.MatmulPerfMode.DoubleRow
    Copy = mybir.ActivationFunctionType.Copy
    Tanh = mybir.ActivationFunctionType.Tanh
    Ident = mybir.ActivationFunctionType.Identity
    mult = mybir.AluOpType.mult
    add = mybir.AluOpType.add

    def dma_zut(t):
        z = iop.tile([ZU, B], f32r, tag="zut", name=f"zut_{t}")
        nc.sync.dma_start(z[:], zut_d[:, t * B:(t + 1) * B])
        return z

    def emit_A_chunk(t, j, zts, h1t):
        """pa_{t,j} = W1 @ z_t ; h1_t[:,j,:] = tanh(pa + b1). PE + ACT."""
        pa = smp.tile([128, B], f32, tag="sm", name=f"pa_{t}_{j}")
        nc.tensor.matmul(pa[:], w1t[:, j * 128:(j + 1) * 128], zts[0:D2, :],
                         start=True, stop=True)
        nc.scalar.activation(h1t[:, j, :], pa[:], Tanh, bias=b1c[:, j:j + 1])

    def emit_pb(t, j, h1t):
        """a2 pre-activation matmul group for chunk j (8 accumulating mms)."""
        pb = mmp.tile([128, B], f32, tag="mm", name=f"pb_{t}_{j}")
        for k in range(HC):
            nc.tensor.matmul(pb[:], w2t[:, k, j * 128:(j + 1) * 128],
                             h1t[:, k, :], start=(k == 0), stop=(k == 7))
        return pb

    # ---- resident activation tensors ----
    s1 = actp.tile([128, HC, B], f32r, tag="s1", name="s1")
    s18 = actp.tile([128, HC, B], fp8, tag="s18", name="s18")
    s0 = actp.tile([128, HC, B], f32, tag="s0", name="s0")
    g1 = actp.tile([128, HC, B], bf16, tag="g1", name="g1")

    # ---- prologue: tile 0 layer-1 forward (zut0/zut1 pre-DMA'd) ----
    zuts = {0: zut0, 1: zut1}
    h1s = {0: h1p.tile([128, HC, B], bf16, tag="h1", name="h1_0")}
    for j in range(HC):
        emit_A_chunk(0, j, zuts[0], h1s[0])

    pb0_next = None
    for t in range(TILES):
        sl = slice(t * B, (t + 1) * B)
        zut = zuts.pop(t)
        h1 = h1s.pop(t)
        if t + 2 < TILES:
            zuts[t + 2] = dma_zut(t + 2)
        if t + 1 < TILES:
            h1s[t + 1] = h1p.tile([128, HC, B], bf16, tag="h1",
                                  name=f"h1_{t + 1}")

        # ---- BC: a2 -> s1 ; f1 -> g1 ----
        for j in range(HC):
            pb = pb0_next if (j == 0 and pb0_next is not None) \
                else emit_pb(t, j, h1)
            h2 = tmp2.tile([128, B], f32, tag="h2", name=f"h2_{t}_{j}")
            nc.scalar.activation(h2[:], pb[:], Tanh, bias=b2c[:, j:j + 1])
            nc.vector.tensor_tensor(out=h2[:], in0=h2[:], in1=h2[:], op=mult)
            nc.vector.tensor_scalar(out=s1[:, j, :], in0=h2[:],
                                    scalar1=-1.0, scalar2=1.0,
                                    op0=mult, op1=add)
            nc.scalar.activation(s18[:, j, :], s1[:, j, :], Copy, scale=128.0)

            pf = smp.tile([128, B], f32, tag="sm", name=f"pf_{t}_{j}")
            nc.tensor.matmul(pf[:], wf1t[:, j * 128:(j + 1) * 128], zut[:],
                             start=True, stop=True)
            th = tmpp.tile([128, B], f32, tag="th", name=f"th_{t}_{j}")
            nc.scalar.activation(th[:], pf[:], Tanh, bias=bf1c[:, j:j + 1])
            prod = tmpp.tile([128, B], f32, tag="prod", name=f"prod_{t}_{j}")
            acc = tmp2.tile([128, 1], f32, tag="acc", name=f"acc_{t}_{j}")
            nc.vector.affine_mul_reduce(prod[:], acc[:], pf[:], s1[:, j, :],
                                        1.0, bf1c[:, j:j + 1])
            nc.gpsimd.tensor_tensor(out=g1[:, j, :], in0=th[:], in1=prod[:],
                                    op=add)
        pb0_next = None

        # ---- seam: start next tile's layer-1 (covers s1_7 ACT/DVE latency) --
        if t + 1 < TILES:
            for j in range(2):
                emit_A_chunk(t + 1, j, zuts[t + 1], h1s[t + 1])

        # ---- D: gh -> ga1 -> dH accum ; lazy s0 ; rest of next layer-1 ----
        # gh runs as fp8 DoubleRow: 4 matmuls of K=256 (chunk pairs). The
        # 2^22 fp8 scaling (w2w x2^14, s1 x2^8) is undone host-side in w1n.
        ps = accp.tile([D2, B], f32, tag="acc", name=f"ps_{t}")
        ga1s = []
        for k in range(HC):
            pg = mmp.tile([128, B], f32, tag="mm", name=f"pg_{t}_{k}")
            for pr in range(HC // 2):
                nc.tensor.matmul(pg[:],
                                 w2w[:, 2 * pr:2 * pr + 2,
                                     k * 128:(k + 1) * 128],
                                 s18[:, 2 * pr:2 * pr + 2, :],
                                 start=(pr == 0), stop=(pr == 3),
                                 perf_mode=DoubleRow)
            sq = tmp2.tile([128, B], bf16, tag="sq", name=f"sq_{t}_{k}")
            nc.vector.tensor_tensor(out=sq[:], in0=h1[:, k, :], in1=h1[:, k, :],
                                    op=mult)
            nc.vector.tensor_scalar(out=s0[:, k, :], in0=sq[:],
                                    scalar1=-1.0, scalar2=1.0,
                                    op0=mult, op1=add)
            ga1 = tmp2.tile([128, B], f32r, tag="ga1", name=f"ga1_{t}_{k}")
            nc.vector.tensor_tensor(out=ga1[:], in0=pg[:], in1=s0[:, k, :],
                                    op=mult)
            ga1s.append(ga1)
            if k >= 2:
                nc.tensor.matmul(ps[:], w1n[:, k - 2, :], ga1s[k - 2][:],
                                 start=(k == 2), stop=False)
            if t + 1 < TILES and k < HC - 2:
                emit_A_chunk(t + 1, 2 + k, zuts[t + 1], h1s[t + 1])

        # ---- EF: f2 -> g2 -> head accum (pipelined 1 behind; g2 on DVE).
        # pf2_0 is emitted before the two trailing dH accumulations so the
        # PE isn't waiting on ga1_7's elementwise chain.
        g2s = []
        for j in range(HC):
            pf2 = mmp.tile([128, B], f32, tag="mm", name=f"pf2_{t}_{j}")
            for k in range(HC):
                nc.tensor.matmul(pf2[:], wf2t[:, k, j * 128:(j + 1) * 128],
                                 g1[:, k, :], start=(k == 0), stop=(k == 7))
            if j == 0:
                nc.tensor.matmul(ps[:], w1n[:, HC - 2, :], ga1s[HC - 2][:],
                                 start=False, stop=False)
                nc.tensor.matmul(ps[:], w1n[:, HC - 1, :], ga1s[HC - 1][:],
                                 start=False, stop=False)
            th2 = tmp2.tile([128, B], f32, tag="th", name=f"th2_{t}_{j}")
            nc.scalar.activation(th2[:], pf2[:], Tanh, bias=bf2c[:, j:j + 1])
            prod2 = tmpp.tile([128, B], f32, tag="prod", name=f"prod2_{t}_{j}")
            acc2 = tmp2.tile([128, 1], f32, tag="acc", name=f"acc2_{t}_{j}")
            nc.vector.affine_mul_reduce(prod2[:], acc2[:], pf2[:], s0[:, j, :],
                                        1.0, bf2c[:, j:j + 1])
            g2 = tmp2.tile([128, B], f32r, tag="g2", name=f"g2_{t}_{j}")
            nc.vector.tensor_tensor(out=g2[:], in0=th2[:], in1=prod2[:], op=add)
            g2s.append(g2)
            if j >= 1:
                nc.tensor.matmul(ps[:], wfft[:, j - 1, :], g2s[j - 1][:],
                                 start=False, stop=False)

        # hoist next tile's first a2 matmul group ahead of the trailing head
        # accumulation so the PE isn't waiting on g2_7's elementwise chain
        if t + 1 < TILES:
            pb0_next = emit_pb(t + 1, 0, h1s[t + 1])
        nc.tensor.matmul(ps[:], wfft[:, HC - 1, :], g2s[HC - 1][:],
                         start=False, stop=True)

        sout = outp.tile([D2, B], f32, tag="sout", name=f"sout_{t}")
        nc.scalar.activation(sout[:], ps[:], Ident, bias=bffc[:, 0:1])
        nc.sync.dma_start(st_d[:, sl], sout[:])


def _prep_inputs(t, z, W1, b1, W2, b2, Wh, bh, Wf1, bf1, Wf2, bf2, Wff, bff,
                 Wp, bp):
    f = np.float32
    z = np.asarray(z, f)
    u = np.tanh(z @ np.asarray(Wp, f).T + np.asarray(bp, f))
    zu = np.concatenate([z, u], axis=1)          # [N, 20]

    def c(x):
        return np.ascontiguousarray(np.asarray(x, f))

    import ml_dtypes
    bf16 = ml_dtypes.bfloat16
    fp8 = ml_dtypes.float8_e4m3
    W2 = np.asarray(W2, f)
    wh = np.asarray(Wh, f).reshape(-1, 1)        # [H, 1]

    # gh GEMM runs in fp8 (DoubleRow): scale W2w by 2^14 and s1 by 2^7 on
    # chip (e4m3 max finite 240); the 2^-21 descale is folded into w1n here.
    shared = {
        "w1t": c(np.asarray(W1, f).T),
        "w1n": c(np.asarray(W1, f) * np.float32(2.0 ** -21)),
        "w2t": np.ascontiguousarray(W2.T.astype(bf16)),
        "w2w": np.ascontiguousarray(
            ((W2 * wh) * np.float32(2.0 ** 14)).astype(fp8)),
        "wf1t": c(np.asarray(Wf1, f).T),
        "wf2t": np.ascontiguousarray(np.asarray(Wf2, f).T.astype(bf16)),
        "wfft": c(np.asarray(Wff, f).T),
        "b1c": c(np.asarray(b1, f).reshape(HC, 128).T),
        "b2c": c(np.asarray(b2, f).reshape(HC, 128).T),
        "bf1c": c(np.asarray(bf1, f).reshape(HC, 128).T),
        "bf2c": c(np.asarray(bf2, f).reshape(HC, 128).T),
        "bffc": c(np.asarray(bff, f).reshape(D2, 1)),
    }
    in_maps = []
    for r in range(NCORES):
        m = dict(shared)
        m["zut"] = c(zu[r * NSH:(r + 1) * NSH].T)
        in_maps.append(m)
    return in_maps


def _postprocess(results):
    outs = []
    for r in range(NCORES):
        s = results[r]["st"].T                    # [NSH, 16]
        outs.append(np.concatenate([s[:, DQ:], -s[:, :DQ]], axis=1))
    return np.ascontiguousarray(np.concatenate(outs, axis=0).astype(np.float32))


def kernel(**inputs):
    global _BUILT
    from concourse.bass_utils import run_bass_kernel_spmd

    if _BUILT is None:
        _BUILT = _build()
    in_maps = _prep_inputs(**inputs)
    res = run_bass_kernel_spmd(_BUILT, in_maps, list(range(NCORES)))
    return _postprocess(res.results)
